# revision 35
# baseline (speedup 1.0000x reference)
"""Trainium2 Bass kernel for a 16-head MHA block (B=1, S=4096, H=1024).

Sharding: tensor-parallel over heads — each of the 8 cores owns 2 heads
(128 of the 1024 Wq/Wk/Wv output channels) and computes 512 rows of the
final (scrambled) output; the host concatenates the row blocks.

Per-core dataflow:
  qT/kT = relu(W @ x.T + b)      fp8e4m3 DoubleRow matmuls (K=256/chunk),
                                 bf16 [128 chan, 4096 seq] outputs
  v     = relu(x @ W.T + b)      bf16 matmuls, stored e4m3 in DoubleRow
                                 layout [128 t, chunk16, head2, par2, 80]
  S_T[t,s] = sum_d kT[d,t] qT[d,s]   bf16, 2 heads row-tiled (T0/T8, K=64
                                     each) so both matmuls run concurrently
  E = exp(S_T/8) in fp8e4m3, split across engines BY HEAD:
      head0: VectorE Schraudolph bit-trick (fp32 PSUM -> int8 e4m3 bits)
      head1: ScalarE exact Exp with fp8 output
  num/den = sum_t [v|1][t,d'] E[t,s]   fp8 DoubleRow matmul, K=256 virtual
            (t-chunk pairs), row 64 of v = 1.0 gives the denominator
  epilogue: den copy on ScalarE, recip + normalize + residual on DVE,
  partition-broadcast on GpSimd, interleaved as side-tasks into the
  next s-block's iteration loop.
"""

import math

import numpy as np
import ml_dtypes

import concourse.bass as bass
import concourse.tile as tile
from concourse import bacc, mybir
from concourse.bass import ds, ts
from concourse.bass_utils import run_bass_kernel_spmd

BF16 = ml_dtypes.bfloat16
FP8 = ml_dtypes.float8_e4m3
S = 4096
H = 1024
NCORES = 8
OC = H // NCORES  # 128 output channels (2 heads) per core
SBLK = 512  # s-block width
NSB = S // SBLK  # 8
NT = S // 128  # 32 t-chunks of 128
NC2 = NT // 2  # 16 DoubleRow t-chunks of 256
NKC = H // 256  # 4 DoubleRow contraction chunks for q/k projections
NKCV = H // 128  # 8 plain contraction chunks for the v projection

# Schraudolph exp for the DVE share, to e4m3 bits: bits of exp(s/8) are
# approx round(s/ln2 + 8*(7 - 0.0437)); scores >= 0 (post-relu q,k).
SCH8_MUL = 1.0 / math.log(2.0)
SCH8_ADD = 8.0 * (7.0 - 0.0437)

_CACHE = {}


def _build_nc():
    f32 = mybir.dt.float32
    bf16 = mybir.dt.bfloat16
    fp8 = mybir.dt.float8e4
    i8 = mybir.dt.int8
    add = mybir.AluOpType.add
    mult = mybir.AluOpType.mult
    Exp = mybir.ActivationFunctionType.Exp
    Relu = mybir.ActivationFunctionType.Relu
    DR = mybir.MatmulPerfMode.DoubleRow

    nc = bacc.Bacc("TRN2", target_bir_lowering=False, debug=False)

    # inputs are host-pre-arranged so every DMA is contiguous per partition:
    # xq/xk: [sb, p, kc, i, s_local] fp8 (chan = 256*kc + 128*i + p)
    # xv:    [sb, p, c, s_local] bf16 (chan = 128*c + p)
    # wq/wk: [p, kc, i, o] fp8 ; wv: [p, c, o] bf16
    xq_r = nc.dram_tensor(
        "xq", [NSB, 128, NKC, 2, SBLK], fp8, kind="ExternalInput"
    ).ap()
    xk_r = nc.dram_tensor(
        "xk", [NSB, 128, NKC, 2, SBLK], fp8, kind="ExternalInput"
    ).ap()
    xv_r = nc.dram_tensor(
        "xv", [NSB, 128, NKCV, SBLK], bf16, kind="ExternalInput"
    ).ap()
    wq_r = nc.dram_tensor("wq", [128, NKC, 2, OC], fp8, kind="ExternalInput").ap()
    wk_r = nc.dram_tensor("wk", [128, NKC, 2, OC], fp8, kind="ExternalInput").ap()
    wv_r = nc.dram_tensor("wv", [128, NKCV, OC], bf16, kind="ExternalInput").ap()
    bq = nc.dram_tensor("bq", [OC, 1], f32, kind="ExternalInput").ap()
    bk = nc.dram_tensor("bk", [OC, 1], f32, kind="ExternalInput").ap()
    bv = nc.dram_tensor("bv", [1, OC], bf16, kind="ExternalInput").ap()
    qres = nc.dram_tensor("qres", [512, H], bf16, kind="ExternalInput").ap()
    out = nc.dram_tensor("out", [512, H], bf16, kind="ExternalOutput").ap()
    # residual/output rows: local row = 256*hl + 4*d + j
    qres_r = qres.rearrange("(hl d j) m -> hl d j m", hl=2, d=64)
    out_r = out.rearrange("(hl d j) m -> hl d j m", hl=2, d=64)

    with tile.TileContext(nc) as tc:
        with (
            tc.tile_pool(name="const", bufs=1) as constp,
            tc.tile_pool(name="persist", bufs=1) as persist,
            tc.tile_pool(name="stage", bufs=2) as stage,
            tc.tile_pool(name="exps", bufs=6) as expp,
            tc.tile_pool(name="epi", bufs=2) as epi,
            tc.tile_pool(name="ps_sc", bufs=2, space="PSUM") as psc,
            tc.tile_pool(name="ps_av", bufs=2, space="PSUM") as ps_av,
        ):
            # ---- constants ----
            wq_sb = constp.tile([128, NKC, 2, OC], fp8)
            wk_sb = constp.tile([128, NKC, 2, OC], fp8)
            wv_sb = constp.tile([128, NKCV, OC], bf16)
            nc.sync.dma_start(wk_sb[:], wk_r)
            bq_sb = constp.tile([OC, 1], f32)
            bk_sb = constp.tile([OC, 1], f32)
            bv_sb = constp.tile([1, OC], bf16)
            nc.scalar.dma_start(bq_sb[:], bq)
            nc.scalar.dma_start(bk_sb[:], bk)
            nc.scalar.dma_start(bv_sb[:], bv)
            ones_rowb = constp.tile([1, 128], bf16)
            nc.vector.memset(ones_rowb[:], 1.0)

            qT_sb = persist.tile([128, S], bf16)
            kT_sb = persist.tile([128, S], bf16)
            # v in fp8 DoubleRow layout: [p, chunk, head, i, 80]
            # element = v[t = 256*chunk + 128*i + p, 64*head + d]; d=64 is
            # the ones column (denominator row of the AV matmul).
            v_sb = persist.tile([128, NC2, 2, 2, 80], fp8)
            nc.vector.memset(v_sb[:, :, :, :, 64:65], 1.0)

            # ---- helper defs ----
            side_sched = []  # [slot, fn]: fn runs at first iteration >= slot

            def q_proj(sb, xq_st=None):
                ss = ds(sb * SBLK, SBLK)
                if xq_st is None:
                    xq_st = stage.tile(
                        [128, NKC, 2, SBLK], fp8, tag="xq", name="xq_st"
                    )
                    nc.sync.dma_start(xq_st[:], xq_r[sb])
                # qp lives in the av bank-pair of the s-block being
                # epilogued (dead by the time q_proj runs)
                qp = ps_av.tile([128, 1024], f32, tag=f"av{sb % 2}", bufs=1, name="qp")
                for kc in range(NKC):
                    nc.tensor.matmul(
                        qp[:, :SBLK], wq_sb[:, kc, :, :], xq_st[:, kc, :, :],
                        start=(kc == 0), stop=(kc == NKC - 1), perf_mode=DR,
                    )
                nc.scalar.activation(
                    qT_sb[:, ss], qp[:, :SBLK], Relu, bias=bq_sb[:]
                )

            def q_proj_sched(sb, slots):
                # split into matmul halves + relu, spaced so nothing waits
                # at its engine-queue head
                ss = ds(sb * SBLK, SBLK)
                xq_st = stage.tile([128, NKC, 2, SBLK], fp8, tag="xq", name="xq_st")
                nc.sync.dma_start(xq_st[:], xq_r[sb])
                st = {}

                def mm(c0, c1):
                    def f():
                        if "qp" not in st:
                            st["qp"] = ps_av.tile(
                                [128, 1024], f32, tag=f"av{sb % 2}", bufs=1, name="qp"
                            )
                        for kc in range(c0, c1):
                            nc.tensor.matmul(
                                st["qp"][:, :SBLK], wq_sb[:, kc, :, :],
                                xq_st[:, kc, :, :],
                                start=(kc == 0), stop=(kc == NKC - 1),
                                perf_mode=DR,
                            )
                    return f

                def rl():
                    nc.scalar.activation(
                        qT_sb[:, ss], st["qp"][:, :SBLK], Relu, bias=bq_sb[:]
                    )

                side_sched.extend(
                    [[slots[0], mm(0, 2)], [slots[1], mm(2, 4)], [slots[2], rl]]
                )

            def scores_exp(sb, ti, ex4, warm=False):
                # ex4: [128, head, i, 512] e4m3 tile for chunk ti//2.
                # Both heads' scores land in ONE 2-bank tile (h0 -> bank A,
                # h1 -> bank B; the row-tiled MMs stay concurrent).
                ss = ds(sb * SBLK, SBLK)
                tt = ds(ti * 128, 128)
                i = ti % 2
                sc = psc.tile([128, 1024], f32, tag="sc", name="sc")
                nc.tensor.matmul(
                    sc[:, 0:512], kT_sb[0:64, tt], qT_sb[0:64, ss],
                    start=True, stop=True,
                )
                nc.tensor.matmul(
                    sc[:, 512:1024], kT_sb[64:128, tt], qT_sb[64:128, ss],
                    start=True, stop=True,
                )
                if warm:
                    # warmup tiles are single-buffered: split by head so
                    # both engines work in parallel (shorter dep chain)
                    nc.vector.tensor_scalar(
                        ex4.bitcast(i8)[:, 0, i, :], sc[:, 0:512],
                        SCH8_MUL, SCH8_ADD, mult, add,
                    )
                    nc.scalar.activation(
                        ex4[:, 1, i, :], sc[:, 512:1024], Exp, scale=0.125
                    )
                    return
                sc_r = sc.rearrange("p (h s) -> p h s", h=2)
                if ti % 2 == 0:
                    nc.scalar.activation(
                        ex4[:, :, i, :], sc_r, Exp, scale=0.125
                    )
                else:
                    nc.vector.tensor_scalar(
                        ex4.bitcast(i8)[:, :, i, :], sc_r,
                        SCH8_MUL, SCH8_ADD, mult, add,
                    )

            def av_mm(av, c, ex4):
                for hl in range(2):
                    nc.tensor.matmul(
                        av[0:65, ts(hl, SBLK)],
                        v_sb[:, c, hl, :, 0:65],
                        ex4[:, hl, :, :],
                        start=(c == 0), stop=(c == NC2 - 1),
                        perf_mode=DR,
                    )

            pend = []  # deferred (chunk, ex4) AV inputs
            cur = []  # ex4 tile being filled (allocated at even ti)

            def attn_ti(sb, av, ti, warm=False):
                if ti % 2 == 0:
                    cur.append(expp.tile([128, 2, 2, 512], fp8, name="ex4"))
                ex4 = cur[-1]
                scores_exp(sb, ti, ex4, warm=warm)
                if ti % 2 == 1:
                    # chunk complete; keep AV one chunk behind the scores so
                    # the AV matmuls never wait on freshly-produced ex
                    if pend:
                        pc, pex = pend.pop(0)
                        av_mm(av, pc, pex)
                    pend.append((ti // 2, ex4))
                    cur.clear()
                    if ti == NT - 1:
                        for pc, pex in pend:
                            av_mm(av, pc, pex)
                        pend.clear()
                # slot-scheduled side work (epilogue pieces, next q_proj)
                side_sched.sort(key=lambda x: x[0])
                while side_sched and ti >= side_sched[0][0]:
                    side_sched.pop(0)[1]()

            def epilogue(sb, av, last=False):
                # normalize + residual + store; split into side-tasks that
                # interleave with the next s-block's iterations. All pieces
                # are scheduled LATE so no engine FIFO head-blocks on the
                # den chain (copy -> DMA -> gpsimd bcast).
                j = sb // 2
                mm = ds((sb % 2) * SBLK, SBLK)
                qrt = epi.tile([64, 1024], bf16, name="qrt")
                for hl in range(2):
                    nc.gpsimd.dma_start(
                        qrt[:, ts(hl, SBLK)], qres_r[hl, :, j, mm]
                    )
                d64 = epi.tile([65, 1024], f32, name="d64")
                den0 = epi.tile([1, 1024], f32, name="den0")
                bcd = epi.tile([64, 1024], f32, name="bcd")
                bcs = epi.tile([64, 1024], f32, name="bcs")
                prod = epi.tile([64, 1024], bf16, name="prod")
                outt = epi.tile([64, 1024], bf16, name="outt")

                def dn():
                    # den row (partition 64) -> SBUF -> partition 0 -> bcast
                    nc.scalar.copy(d64[64:65, :], av[64:65, :])
                    nc.sync.dma_start(den0[:], d64[64:65, :])
                    nc.gpsimd.partition_broadcast(bcd[:], den0[:])

                def rc():
                    nc.vector.reciprocal_approx_fast(bcs[:], bcd[:])

                def pr():
                    nc.vector.tensor_tensor(
                        prod[:], av[0:64, :], bcs[:], mult
                    )

                def fin():
                    eng = nc.vector if last else nc.gpsimd
                    eng.tensor_tensor(outt[:], prod[:], qrt[:], add)
                    for hl in range(2):
                        nc.gpsimd.dma_start(
                            out_r[hl, :, j, mm], outt[:, ts(hl, SBLK)]
                        )

                side_sched.extend([[1, dn], [9, rc], [11, pr], [13, fin]])

            # ---- k/v projections interleaved with attention(0) ----
            av0 = ps_av.tile([128, 1024], f32, tag="av0", bufs=1, name="av")
            # kp lives in the av1 bank-pair, idle until s-block 1 starts
            av1_wu = ps_av.tile([128, 1024], f32, tag="av1", bufs=1, name="av1_wu")
            kp = av1_wu[:, 512:1024]

            def k_proj(sb, xk_st):
                # k projection runs one s-chunk AHEAD of attention(0) so
                # the k-relu is never on the scores' critical path
                for kc in range(NKC):
                    nc.tensor.matmul(
                        kp, wk_sb[:, kc, :, :], xk_st[:, kc, :, :],
                        start=(kc == 0), stop=(kc == NKC - 1), perf_mode=DR,
                    )
                nc.scalar.activation(
                    kT_sb[:, ds(sb * SBLK, SBLK)], kp, Relu, bias=bk_sb[:]
                )

            def dma_xk(sb):
                xk_st = stage.tile([128, NKC, 2, SBLK], fp8, tag="xk", name="xk_st")
                nc.sync.dma_start(xk_st[:], xk_r[sb])
                return xk_st

            def dma_xv(sb):
                xv_st = stage.tile([128, NKCV, SBLK], bf16, tag="xv", name="xv_st")
                nc.sync.dma_start(xv_st[:], xv_r[sb])
                return xv_st

            xk_cur = dma_xk(0)
            xv_cur = dma_xv(0)
            xq0_st = stage.tile([128, NKC, 2, SBLK], fp8, tag="xq", name="xq_st")
            nc.sync.dma_start(xq0_st[:], xq_r[0])
            xq1_st = stage.tile([128, NKC, 2, SBLK], fp8, tag="xq", name="xq_st")
            nc.sync.dma_start(xq1_st[:], xq_r[1])
            nc.sync.dma_start(wv_sb[:], wv_r)
            nc.sync.dma_start(wq_sb[:], wq_r)
            k_proj(0, xk_cur)
            for sb in range(NSB):
                if sb + 1 < NSB:
                    xk_nxt = dma_xk(sb + 1)
                    xv_nxt = dma_xv(sb + 1)
                for tj in range(4):
                    ti = sb * 4 + tj
                    vp = psc.tile([128, 1024], f32, tag="sc", name="vp")
                    for ci in range(NKCV):
                        nc.tensor.matmul(
                            vp[:, 0:128], xv_cur[:, ci, ts(tj, 128)], wv_sb[:, ci, :],
                            start=(ci == 0), stop=False,
                        )
                    nc.tensor.matmul(
                        vp[:, 0:128], ones_rowb[:1, :], bv_sb[:1, :],
                        start=False, stop=True,
                    )
                    nc.vector.tensor_scalar_max(
                        v_sb[:, ti // 2, :, ti % 2, 0:64],
                        vp[:, 0:128].rearrange("p (h w) -> p h w", h=2),
                        0.0,
                    )
                    if sb == 0 and tj == 0:
                        q_proj(0, xq_st=xq0_st)
                    if tj == 2 and sb + 1 < NSB:
                        k_proj(sb + 1, xk_nxt)
                    attn_ti(0, av0, ti, warm=True)
                if sb + 1 < NSB:
                    xk_cur, xv_cur = xk_nxt, xv_nxt
            epilogue(0, av0)

            # ---- remaining attention s-blocks ----
            q_proj(1, xq_st=xq1_st)
            for sb in range(1, NSB):
                if sb + 1 < NSB:
                    q_proj_sched(sb + 1, (12, 13, 14))
                av = ps_av.tile([128, 1024], f32, tag=f"av{sb % 2}", bufs=1, name="av")
                for ti in range(NT):
                    attn_ti(sb, av, ti)
                epilogue(sb, av, last=(sb == NSB - 1))
            for _, fn in sorted(side_sched, key=lambda x: x[0]):
                fn()
            side_sched.clear()

    nc.compile()
    return nc


def _get_nc():
    if "nc" not in _CACHE:
        _CACHE["nc"] = _build_nc()
    return _CACHE["nc"]


def _arr_x8(x2):
    # [S, H] -> [sb, p, kc, i, s_local] fp8, chan = 256*kc + 128*i + p
    xT = x2.T.astype(FP8)  # [H, S]
    return np.ascontiguousarray(
        xT.reshape(NKC, 2, 128, NSB, SBLK).transpose(3, 2, 0, 1, 4)
    )


def _arr_xv(x2):
    # [S, H] -> [sb, p, c, s_local] bf16, chan = 128*c + p
    xT = x2.T.astype(BF16)  # [H, S]
    return np.ascontiguousarray(
        xT.reshape(NKCV, 128, NSB, SBLK).transpose(2, 1, 0, 3)
    )


def _arr_w8(wT):
    # [H, OC] -> [p, kc, i, o] fp8
    return np.ascontiguousarray(
        wT.astype(FP8).reshape(NKC, 2, 128, OC).transpose(2, 0, 1, 3)
    )


def _arr_wv(wT):
    # [H, OC] -> [p, c, o] bf16
    return np.ascontiguousarray(
        wT.astype(BF16).reshape(NKCV, 128, OC).transpose(1, 0, 2)
    )


def kernel(queries, keys, values, Wq_w, Wq_b, Wk_w, Wk_b, Wv_w, Wv_b, **kw):
    nc = _get_nc()
    q2 = np.asarray(queries, np.float32).reshape(S, H)
    k2 = np.asarray(keys, np.float32).reshape(S, H)
    v2 = np.asarray(values, np.float32).reshape(S, H)
    xq4 = _arr_x8(q2)
    xk4 = _arr_x8(k2)
    xv4 = _arr_xv(v2)

    in_maps = []
    for c in range(NCORES):
        o = slice(OC * c, OC * (c + 1))
        in_maps.append(
            {
                "xq": xq4,
                "xk": xk4,
                "xv": xv4,
                "wq": _arr_w8(np.asarray(Wq_w)[o].T),
                "wk": _arr_w8(np.asarray(Wk_w)[o].T),
                "wv": _arr_wv(np.asarray(Wv_w)[o].T),
                "bq": np.asarray(Wq_b, np.float32)[o].reshape(OC, 1),
                "bk": np.asarray(Wk_b, np.float32)[o].reshape(OC, 1),
                "bv": np.asarray(Wv_b)[o].astype(BF16).reshape(1, OC),
                "qres": np.ascontiguousarray(
                    q2[512 * c : 512 * (c + 1)]
                ).astype(BF16),
            }
        )

    res = run_bass_kernel_spmd(
        nc, in_maps, list(range(NCORES)), **_CACHE.get("run_kwargs", {})
    )
    _CACHE["last_results"] = res
    full = np.concatenate(
        [np.asarray(res.results[c]["out"], np.float32) for c in range(NCORES)],
        axis=0,
    )
    return full.reshape(1, S, H)


# revision 36
# speedup vs baseline: 1.0910x; 1.0910x over previous
"""Trainium2 Bass kernel for a 16-head MHA block (B=1, S=4096, H=1024).

Sharding: tensor-parallel over heads — each of the 8 cores owns 2 heads
(128 of the 1024 Wq/Wk/Wv output channels) and computes 512 rows of the
final (scrambled) output; the host concatenates the row blocks.

Per-core dataflow:
  qT/kT = relu(W @ x.T + b)      fp8e4m3 DoubleRow matmuls (K=256/chunk),
                                 bf16 [128 chan, 4096 seq] outputs
  v     = relu(x @ W.T + b)      bf16 matmuls, stored e4m3 in DoubleRow
                                 layout [128 t, chunk16, head2, par2, 80]
  S_T[t,s] = sum_d kT[d,t] qT[d,s]   bf16, 2 heads row-tiled (T0/T8, K=64
                                     each) so both matmuls run concurrently
  E = exp(S_T/8) in fp8e4m3, split across engines BY HEAD:
      head0: VectorE Schraudolph bit-trick (fp32 PSUM -> int8 e4m3 bits)
      head1: ScalarE exact Exp with fp8 output
  num/den = sum_t [v|1][t,d'] E[t,s]   fp8 DoubleRow matmul, K=256 virtual
            (t-chunk pairs), row 64 of v = 1.0 gives the denominator
  epilogue: den copy on ScalarE, recip + normalize + residual on DVE,
  partition-broadcast on GpSimd, interleaved as side-tasks into the
  next s-block's iteration loop.
"""

import math

import numpy as np
import ml_dtypes

import concourse.bass as bass
import concourse.tile as tile
from concourse import bacc, mybir
from concourse.bass import ds, ts
from concourse.bass_utils import run_bass_kernel_spmd

BF16 = ml_dtypes.bfloat16
FP8 = ml_dtypes.float8_e4m3
S = 4096
H = 1024
NCORES = 8
OC = H // NCORES  # 128 output channels (2 heads) per core
SBLK = 512  # s-block width
NSB = S // SBLK  # 8
NT = S // 128  # 32 t-chunks of 128
NC2 = NT // 2  # 16 DoubleRow t-chunks of 256
NKC = H // 256  # 4 DoubleRow contraction chunks for q/k projections
NKCV = H // 128  # 8 plain contraction chunks for the v projection

# Schraudolph exp for the DVE share, to e4m3 bits: bits of exp(s/8) are
# approx round(s/ln2 + 8*(7 - 0.0437)); scores >= 0 (post-relu q,k).
SCH8_MUL = 1.0 / math.log(2.0)
SCH8_ADD = 8.0 * (7.0 - 0.0437)

_CACHE = {}


def _build_nc():
    f32 = mybir.dt.float32
    bf16 = mybir.dt.bfloat16
    fp8 = mybir.dt.float8e4
    i8 = mybir.dt.int8
    add = mybir.AluOpType.add
    mult = mybir.AluOpType.mult
    Exp = mybir.ActivationFunctionType.Exp
    Relu = mybir.ActivationFunctionType.Relu
    DR = mybir.MatmulPerfMode.DoubleRow

    nc = bacc.Bacc("TRN2", target_bir_lowering=False, debug=False)

    # inputs are host-pre-arranged so every DMA is contiguous per partition:
    # xq/xk: [sb, p, kc, i, s_local] fp8 (chan = 256*kc + 128*i + p)
    # xv:    [sb, p, c, s_local] bf16 (chan = 128*c + p)
    # wq/wk: [p, kc, i, o] fp8 ; wv: [p, c, o] bf16
    xq_r = nc.dram_tensor(
        "xq", [NSB, 128, NKC, 2, SBLK], fp8, kind="ExternalInput"
    ).ap()
    xk_r = nc.dram_tensor(
        "xk", [NSB, 128, NKC, 2, SBLK], fp8, kind="ExternalInput"
    ).ap()
    xv_r = nc.dram_tensor(
        "xv", [NSB, 128, NKCV, SBLK], bf16, kind="ExternalInput"
    ).ap()
    wq_r = nc.dram_tensor("wq", [128, NKC, 2, OC], fp8, kind="ExternalInput").ap()
    wk_r = nc.dram_tensor("wk", [128, NKC, 2, OC], fp8, kind="ExternalInput").ap()
    wv_r = nc.dram_tensor("wv", [128, NKCV, OC], bf16, kind="ExternalInput").ap()
    bq = nc.dram_tensor("bq", [OC, 1], f32, kind="ExternalInput").ap()
    bk = nc.dram_tensor("bk", [OC, 1], f32, kind="ExternalInput").ap()
    bv = nc.dram_tensor("bv", [1, OC], bf16, kind="ExternalInput").ap()
    qres = nc.dram_tensor("qres", [512, H], bf16, kind="ExternalInput").ap()
    out = nc.dram_tensor("out", [512, H], bf16, kind="ExternalOutput").ap()
    # residual/output rows: local row = 256*hl + 4*d + j
    qres_r = qres.rearrange("(hl d j) m -> hl d j m", hl=2, d=64)
    out_r = out.rearrange("(hl d j) m -> hl d j m", hl=2, d=64)

    with tile.TileContext(nc) as tc:
        with (
            tc.tile_pool(name="const", bufs=1) as constp,
            tc.tile_pool(name="persist", bufs=1) as persist,
            tc.tile_pool(name="stage", bufs=2) as stage,
            tc.tile_pool(name="exps", bufs=6) as expp,
            tc.tile_pool(name="epi", bufs=2) as epi,
            tc.tile_pool(name="ps_d", bufs=2, space="PSUM") as ps_d,
            tc.tile_pool(name="ps_s", bufs=2, space="PSUM") as ps_s,
            tc.tile_pool(name="ps_av", bufs=2, space="PSUM") as ps_av,
        ):
            # ---- constants ----
            wq_sb = constp.tile([128, NKC, 2, OC], fp8)
            wk_sb = constp.tile([128, NKC, 2, OC], fp8)
            wv_sb = constp.tile([128, NKCV, OC], bf16)
            nc.sync.dma_start(wk_sb[:], wk_r)
            bq_sb = constp.tile([OC, 1], f32)
            bk_sb = constp.tile([OC, 1], f32)
            bv_sb = constp.tile([1, OC], bf16)
            nc.scalar.dma_start(bq_sb[:], bq)
            nc.scalar.dma_start(bk_sb[:], bk)
            nc.scalar.dma_start(bv_sb[:], bv)
            ones_rowb = constp.tile([1, 128], bf16)
            nc.vector.memset(ones_rowb[:], 1.0)

            qT_sb = persist.tile([128, S], bf16)
            kT_sb = persist.tile([128, S], bf16)
            # v in fp8 DoubleRow layout: [p, chunk, head, i, 80]
            # element = v[t = 256*chunk + 128*i + p, 64*head + d]; d=64 is
            # the ones column (denominator row of the AV matmul).
            v_sb = persist.tile([128, NC2, 2, 2, 80], fp8)
            nc.vector.memset(v_sb[:, :, :, :, 64:65], 1.0)

            # ---- helper defs ----
            side_sched = []  # [slot, fn]: fn runs at first iteration >= slot

            def q_proj(sb, xq_st=None):
                ss = ds(sb * SBLK, SBLK)
                if xq_st is None:
                    xq_st = stage.tile(
                        [128, NKC, 2, SBLK], fp8, tag="xq", name="xq_st"
                    )
                    nc.sync.dma_start(xq_st[:], xq_r[sb])
                # qp lives in the av bank-pair of the s-block being
                # epilogued (dead by the time q_proj runs)
                qp = ps_av.tile([128, 1024], f32, tag=f"av{sb % 2}", bufs=1, name="qp")
                for kc in range(NKC):
                    nc.tensor.matmul(
                        qp[:, :SBLK], wq_sb[:, kc, :, :], xq_st[:, kc, :, :],
                        start=(kc == 0), stop=(kc == NKC - 1), perf_mode=DR,
                    )
                nc.scalar.activation(
                    qT_sb[:, ss], qp[:, :SBLK], Relu, bias=bq_sb[:]
                )

            def q_proj_sched(sb, slots):
                # split into matmul halves + relu, spaced so nothing waits
                # at its engine-queue head
                ss = ds(sb * SBLK, SBLK)
                xq_st = stage.tile([128, NKC, 2, SBLK], fp8, tag="xq", name="xq_st")
                nc.sync.dma_start(xq_st[:], xq_r[sb])
                st = {}

                def mm(c0, c1):
                    def f():
                        if "qp" not in st:
                            st["qp"] = ps_av.tile(
                                [128, 1024], f32, tag=f"av{sb % 2}", bufs=1, name="qp"
                            )
                        for kc in range(c0, c1):
                            nc.tensor.matmul(
                                st["qp"][:, :SBLK], wq_sb[:, kc, :, :],
                                xq_st[:, kc, :, :],
                                start=(kc == 0), stop=(kc == NKC - 1),
                                perf_mode=DR,
                            )
                    return f

                def rl():
                    nc.scalar.activation(
                        qT_sb[:, ss], st["qp"][:, :SBLK], Relu, bias=bq_sb[:]
                    )

                side_sched.extend(
                    [[slots[0], mm(0, 2)], [slots[1], mm(2, 4)], [slots[2], rl]]
                )

            def scores_exp(sb, ti, ex4):
                # ex4: [128, head, i, 512] e4m3 tile for chunk ti//2
                ss = ds(sb * SBLK, SBLK)
                tt = ds(ti * 128, 128)
                i = ti % 2
                scd = ps_d.tile([128, 512], f32, tag="d", name="scd")
                scs = ps_s.tile([128, 512], f32, tag="s", name="scs")
                nc.tensor.matmul(
                    scd[:], kT_sb[0:64, tt], qT_sb[0:64, ss],
                    start=True, stop=True,
                )
                nc.tensor.matmul(
                    scs[:], kT_sb[64:128, tt], qT_sb[64:128, ss],
                    start=True, stop=True,
                )
                nc.vector.tensor_scalar(
                    ex4.bitcast(i8)[:, 0, i, :], scd[:],
                    SCH8_MUL, SCH8_ADD, mult, add,
                )
                nc.scalar.activation(
                    ex4[:, 1, i, :], scs[:], Exp, scale=0.125
                )

            def av_mm(av, c, ex4):
                for hl in range(2):
                    nc.tensor.matmul(
                        av[0:65, ts(hl, SBLK)],
                        v_sb[:, c, hl, :, 0:65],
                        ex4[:, hl, :, :],
                        start=(c == 0), stop=(c == NC2 - 1),
                        perf_mode=DR,
                    )

            pend = []  # deferred (chunk, ex4) AV inputs
            cur = []  # ex4 tile being filled (allocated at even ti)

            def attn_ti(sb, av, ti):
                if ti % 2 == 0:
                    cur.append(expp.tile([128, 2, 2, 512], fp8, name="ex4"))
                ex4 = cur[-1]
                scores_exp(sb, ti, ex4)
                if ti % 2 == 1:
                    # chunk complete; keep AV one chunk behind the scores so
                    # the AV matmuls never wait on freshly-produced ex
                    if pend:
                        pc, pex = pend.pop(0)
                        av_mm(av, pc, pex)
                    pend.append((ti // 2, ex4))
                    cur.clear()
                    if ti == NT - 1:
                        for pc, pex in pend:
                            av_mm(av, pc, pex)
                        pend.clear()
                # slot-scheduled side work (epilogue pieces, next q_proj)
                side_sched.sort(key=lambda x: x[0])
                while side_sched and ti >= side_sched[0][0]:
                    side_sched.pop(0)[1]()

            def epilogue(sb, av, last=False):
                # normalize + residual + store; split into side-tasks that
                # interleave with the next s-block's iterations. All pieces
                # are scheduled LATE so no engine FIFO head-blocks on the
                # den chain (copy -> DMA -> gpsimd bcast).
                j = sb // 2
                mm = ds((sb % 2) * SBLK, SBLK)
                qrt = epi.tile([64, 1024], bf16, name="qrt")
                for hl in range(2):
                    nc.gpsimd.dma_start(
                        qrt[:, ts(hl, SBLK)], qres_r[hl, :, j, mm]
                    )
                d64 = epi.tile([65, 1024], f32, name="d64")
                den0 = epi.tile([1, 1024], f32, name="den0")
                bcd = epi.tile([64, 1024], f32, name="bcd")
                bcs = epi.tile([64, 1024], f32, name="bcs")
                prod = epi.tile([64, 1024], bf16, name="prod")
                outt = epi.tile([64, 1024], bf16, name="outt")

                def dn():
                    # den row (partition 64) -> SBUF -> partition 0 -> bcast
                    nc.scalar.copy(d64[64:65, :], av[64:65, :])
                    nc.sync.dma_start(den0[:], d64[64:65, :])
                    nc.gpsimd.partition_broadcast(bcd[:], den0[:])

                def rc():
                    nc.vector.reciprocal_approx_fast(bcs[:], bcd[:])

                def pr():
                    nc.vector.tensor_tensor(
                        prod[:], av[0:64, :], bcs[:], mult
                    )

                def fin():
                    eng = nc.vector if last else nc.gpsimd
                    eng.tensor_tensor(outt[:], prod[:], qrt[:], add)
                    for hl in range(2):
                        nc.gpsimd.dma_start(
                            out_r[hl, :, j, mm], outt[:, ts(hl, SBLK)]
                        )

                side_sched.extend([[1, dn], [9, rc], [11, pr], [13, fin]])

            # ---- k/v projections interleaved with attention(0) ----
            av0 = ps_av.tile([128, 1024], f32, tag="av0", bufs=1, name="av")
            # kp lives in the av1 bank-pair, idle until s-block 1 starts
            av1_wu = ps_av.tile([128, 1024], f32, tag="av1", bufs=1, name="av1_wu")
            kp = av1_wu[:, 512:1024]

            def k_proj(sb, xk_st):
                # k projection runs one s-chunk AHEAD of attention(0) so
                # the k-relu is never on the scores' critical path
                for kc in range(NKC):
                    nc.tensor.matmul(
                        kp, wk_sb[:, kc, :, :], xk_st[:, kc, :, :],
                        start=(kc == 0), stop=(kc == NKC - 1), perf_mode=DR,
                    )
                nc.scalar.activation(
                    kT_sb[:, ds(sb * SBLK, SBLK)], kp, Relu, bias=bk_sb[:]
                )

            def dma_xk(sb):
                xk_st = stage.tile([128, NKC, 2, SBLK], fp8, tag="xk", name="xk_st")
                nc.sync.dma_start(xk_st[:], xk_r[sb])
                return xk_st

            def dma_xv(sb):
                xv_st = stage.tile([128, NKCV, SBLK], bf16, tag="xv", name="xv_st")
                nc.sync.dma_start(xv_st[:], xv_r[sb])
                return xv_st

            xk_cur = dma_xk(0)
            xv_cur = dma_xv(0)
            xq0_st = stage.tile([128, NKC, 2, SBLK], fp8, tag="xq", name="xq_st")
            nc.sync.dma_start(xq0_st[:], xq_r[0])
            xq1_st = stage.tile([128, NKC, 2, SBLK], fp8, tag="xq", name="xq_st")
            nc.sync.dma_start(xq1_st[:], xq_r[1])
            nc.sync.dma_start(wv_sb[:], wv_r)
            nc.sync.dma_start(wq_sb[:], wq_r)
            k_proj(0, xk_cur)
            for sb in range(NSB):
                if sb + 1 < NSB:
                    xk_nxt = dma_xk(sb + 1)
                    xv_nxt = dma_xv(sb + 1)
                for tj in range(4):
                    ti = sb * 4 + tj
                    vp = ps_d.tile([128, 512], f32, tag="d", name="vp")
                    for ci in range(NKCV):
                        nc.tensor.matmul(
                            vp[:, 0:128], xv_cur[:, ci, ts(tj, 128)], wv_sb[:, ci, :],
                            start=(ci == 0), stop=False,
                        )
                    nc.tensor.matmul(
                        vp[:, 0:128], ones_rowb[:1, :], bv_sb[:1, :],
                        start=False, stop=True,
                    )
                    nc.vector.tensor_scalar_max(
                        v_sb[:, ti // 2, :, ti % 2, 0:64],
                        vp[:, 0:128].rearrange("p (h w) -> p h w", h=2),
                        0.0,
                    )
                    if sb == 0 and tj == 0:
                        q_proj(0, xq_st=xq0_st)
                    if tj == 2 and sb + 1 < NSB:
                        k_proj(sb + 1, xk_nxt)
                    attn_ti(0, av0, ti)
                if sb + 1 < NSB:
                    xk_cur, xv_cur = xk_nxt, xv_nxt
            epilogue(0, av0)

            # ---- remaining attention s-blocks ----
            q_proj(1, xq_st=xq1_st)
            for sb in range(1, NSB):
                if sb + 1 < NSB:
                    q_proj_sched(sb + 1, (12, 13, 14))
                av = ps_av.tile([128, 1024], f32, tag=f"av{sb % 2}", bufs=1, name="av")
                for ti in range(NT):
                    attn_ti(sb, av, ti)
                epilogue(sb, av, last=(sb == NSB - 1))
            for _, fn in sorted(side_sched, key=lambda x: x[0]):
                fn()
            side_sched.clear()

    nc.compile()
    return nc


def _get_nc():
    if "nc" not in _CACHE:
        _CACHE["nc"] = _build_nc()
    return _CACHE["nc"]


def _arr_x8(x2):
    # [S, H] -> [sb, p, kc, i, s_local] fp8, chan = 256*kc + 128*i + p
    xT = x2.T.astype(FP8)  # [H, S]
    return np.ascontiguousarray(
        xT.reshape(NKC, 2, 128, NSB, SBLK).transpose(3, 2, 0, 1, 4)
    )


def _arr_xv(x2):
    # [S, H] -> [sb, p, c, s_local] bf16, chan = 128*c + p
    xT = x2.T.astype(BF16)  # [H, S]
    return np.ascontiguousarray(
        xT.reshape(NKCV, 128, NSB, SBLK).transpose(2, 1, 0, 3)
    )


def _arr_w8(wT):
    # [H, OC] -> [p, kc, i, o] fp8
    return np.ascontiguousarray(
        wT.astype(FP8).reshape(NKC, 2, 128, OC).transpose(2, 0, 1, 3)
    )


def _arr_wv(wT):
    # [H, OC] -> [p, c, o] bf16
    return np.ascontiguousarray(
        wT.astype(BF16).reshape(NKCV, 128, OC).transpose(1, 0, 2)
    )


def kernel(queries, keys, values, Wq_w, Wq_b, Wk_w, Wk_b, Wv_w, Wv_b, **kw):
    nc = _get_nc()
    q2 = np.asarray(queries, np.float32).reshape(S, H)
    k2 = np.asarray(keys, np.float32).reshape(S, H)
    v2 = np.asarray(values, np.float32).reshape(S, H)
    xq4 = _arr_x8(q2)
    xk4 = _arr_x8(k2)
    xv4 = _arr_xv(v2)

    in_maps = []
    for c in range(NCORES):
        o = slice(OC * c, OC * (c + 1))
        in_maps.append(
            {
                "xq": xq4,
                "xk": xk4,
                "xv": xv4,
                "wq": _arr_w8(np.asarray(Wq_w)[o].T),
                "wk": _arr_w8(np.asarray(Wk_w)[o].T),
                "wv": _arr_wv(np.asarray(Wv_w)[o].T),
                "bq": np.asarray(Wq_b, np.float32)[o].reshape(OC, 1),
                "bk": np.asarray(Wk_b, np.float32)[o].reshape(OC, 1),
                "bv": np.asarray(Wv_b)[o].astype(BF16).reshape(1, OC),
                "qres": np.ascontiguousarray(
                    q2[512 * c : 512 * (c + 1)]
                ).astype(BF16),
            }
        )

    res = run_bass_kernel_spmd(
        nc, in_maps, list(range(NCORES)), **_CACHE.get("run_kwargs", {})
    )
    _CACHE["last_results"] = res
    full = np.concatenate(
        [np.asarray(res.results[c]["out"], np.float32) for c in range(NCORES)],
        axis=0,
    )
    return full.reshape(1, S, H)


# revision 38
# speedup vs baseline: 1.1544x; 1.0581x over previous
"""Trainium2 Bass kernel for a 16-head MHA block (B=1, S=4096, H=1024).

Sharding: tensor-parallel over heads — each of the 8 cores owns 2 heads
(128 of the 1024 Wq/Wk/Wv output channels) and computes 512 rows of the
final (scrambled) output; the host concatenates the row blocks.

Per-core dataflow:
  qT/kT = relu(W @ x.T + b)      fp8e4m3 DoubleRow matmuls (K=256/chunk),
                                 bf16 [128 chan, 4096 seq] outputs
  v     = relu(x @ W.T + b)      bf16 matmuls, stored e4m3 in DoubleRow
                                 layout [128 t, chunk16, head2, par2, 80]
  S_T[t,s] = sum_d kT[d,t] qT[d,s]   bf16, 2 heads row-tiled (T0/T8, K=64
                                     each) so both matmuls run concurrently
  E = exp(S_T/8) in fp8e4m3, split across engines BY HEAD:
      head0: VectorE Schraudolph bit-trick (fp32 PSUM -> int8 e4m3 bits)
      head1: ScalarE exact Exp with fp8 output
  num/den = sum_t [v|1][t,d'] E[t,s]   fp8 DoubleRow matmul, K=256 virtual
            (t-chunk pairs), row 64 of v = 1.0 gives the denominator
  epilogue: den copy on ScalarE, recip + normalize + residual on DVE,
  partition-broadcast on GpSimd, interleaved as side-tasks into the
  next s-block's iteration loop.
"""

import math

import numpy as np
import ml_dtypes

import concourse.bass as bass
import concourse.tile as tile
from concourse import bacc, mybir
from concourse.bass import ds, ts
from concourse.bass_utils import run_bass_kernel_spmd

BF16 = ml_dtypes.bfloat16
FP8 = ml_dtypes.float8_e4m3
S = 4096
H = 1024
NCORES = 8
OC = H // NCORES  # 128 output channels (2 heads) per core
SBLK = 512  # s-block width
NSB = S // SBLK  # 8
NT = S // 128  # 32 t-chunks of 128
NC2 = NT // 2  # 16 DoubleRow t-chunks of 256
NKC = H // 256  # 4 DoubleRow contraction chunks for q/k projections
NKCV = H // 128  # 8 plain contraction chunks for the v projection

# Schraudolph exp for the DVE share, to e4m3 bits: bits of exp(s/8) are
# approx round(s/ln2 + 8*(7 - 0.0437)); scores >= 0 (post-relu q,k).
SCH8_MUL = 1.0 / math.log(2.0)
SCH8_ADD = 8.0 * (7.0 - 0.0437)

_CACHE = {}


def _build_nc():
    f32 = mybir.dt.float32
    bf16 = mybir.dt.bfloat16
    fp8 = mybir.dt.float8e4
    i8 = mybir.dt.int8
    add = mybir.AluOpType.add
    mult = mybir.AluOpType.mult
    Exp = mybir.ActivationFunctionType.Exp
    Relu = mybir.ActivationFunctionType.Relu
    DR = mybir.MatmulPerfMode.DoubleRow

    nc = bacc.Bacc("TRN2", target_bir_lowering=False, debug=False)

    # inputs are host-pre-arranged so every DMA is contiguous per partition:
    # xq/xk: [sb, p, kc, i, s_local] fp8 (chan = 256*kc + 128*i + p)
    # xv:    [sb, p, c, s_local] bf16 (chan = 128*c + p)
    # wq/wk: [p, kc, i, o] fp8 ; wv: [p, c, o] bf16
    xq_r = nc.dram_tensor(
        "xq", [NSB, 128, NKC, 2, SBLK], fp8, kind="ExternalInput"
    ).ap()
    xk_r = nc.dram_tensor(
        "xk", [NSB, 128, NKC, 2, SBLK], fp8, kind="ExternalInput"
    ).ap()
    xv_r = nc.dram_tensor(
        "xv", [NSB, 128, NKCV, SBLK], bf16, kind="ExternalInput"
    ).ap()
    wq_r = nc.dram_tensor("wq", [128, NKC, 2, OC], fp8, kind="ExternalInput").ap()
    wk_r = nc.dram_tensor("wk", [128, NKC, 2, OC], fp8, kind="ExternalInput").ap()
    wv_r = nc.dram_tensor("wv", [128, NKCV, OC], bf16, kind="ExternalInput").ap()
    bq = nc.dram_tensor("bq", [OC, 1], f32, kind="ExternalInput").ap()
    bk = nc.dram_tensor("bk", [OC, 1], f32, kind="ExternalInput").ap()
    bv = nc.dram_tensor("bv", [1, OC], bf16, kind="ExternalInput").ap()
    qres = nc.dram_tensor("qres", [512, H], bf16, kind="ExternalInput").ap()
    out = nc.dram_tensor("out", [512, H], bf16, kind="ExternalOutput").ap()
    # residual/output rows: local row = 256*hl + 4*d + j
    qres_r = qres.rearrange("(hl d j) m -> hl d j m", hl=2, d=64)
    out_r = out.rearrange("(hl d j) m -> hl d j m", hl=2, d=64)

    with tile.TileContext(nc) as tc:
        with (
            tc.tile_pool(name="const", bufs=1) as constp,
            tc.tile_pool(name="persist", bufs=1) as persist,
            tc.tile_pool(name="stage", bufs=2) as stage,
            tc.tile_pool(name="exps", bufs=6) as expp,
            tc.tile_pool(name="epi", bufs=2) as epi,
            tc.tile_pool(name="ps_d", bufs=2, space="PSUM") as ps_d,
            tc.tile_pool(name="ps_s", bufs=2, space="PSUM") as ps_s,
            tc.tile_pool(name="ps_av", bufs=2, space="PSUM") as ps_av,
        ):
            # ---- constants ----
            wq_sb = constp.tile([128, NKC, 2, OC], fp8)
            wk_sb = constp.tile([128, NKC, 2, OC], fp8)
            wv_sb = constp.tile([128, NKCV, OC], bf16)
            nc.sync.dma_start(wk_sb[:], wk_r)
            bq_sb = constp.tile([OC, 1], f32)
            bk_sb = constp.tile([OC, 1], f32)
            bv_sb = constp.tile([1, OC], bf16)
            nc.scalar.dma_start(bq_sb[:], bq)
            nc.scalar.dma_start(bk_sb[:], bk)
            nc.scalar.dma_start(bv_sb[:], bv)
            ones_rowb = constp.tile([1, 128], bf16)
            nc.vector.memset(ones_rowb[:], 1.0)
            ones64 = constp.tile([65, 64], bf16)
            nc.vector.memset(ones64[64:65, :], 1.0)

            qT_sb = persist.tile([128, S], bf16)
            kT_sb = persist.tile([128, S], bf16)
            # v in fp8 DoubleRow layout: [p, chunk, head, i, 80]
            # element = v[t = 256*chunk + 128*i + p, 64*head + d]; d=64 is
            # the ones column (denominator row of the AV matmul).
            v_sb = persist.tile([128, NC2, 2, 2, 80], fp8)
            nc.vector.memset(v_sb[:, :, :, :, 64:65], 1.0)

            # ---- helper defs ----
            side_sched = []  # [slot, fn]: fn runs at first iteration >= slot

            def q_proj(sb):
                ss = ds(sb * SBLK, SBLK)
                xq_st = stage.tile([128, NKC, 2, SBLK], fp8, tag="xq", name="xq_st")
                nc.sync.dma_start(xq_st[:], xq_r[sb])
                # qp lives in the av bank-pair of the s-block being
                # epilogued (dead by the time q_proj runs)
                qp = ps_av.tile([128, 1024], f32, tag=f"av{sb % 2}", bufs=1, name="qp")
                for kc in range(NKC):
                    nc.tensor.matmul(
                        qp[:, :SBLK], wq_sb[:, kc, :, :], xq_st[:, kc, :, :],
                        start=(kc == 0), stop=(kc == NKC - 1), perf_mode=DR,
                    )
                nc.scalar.activation(
                    qT_sb[:, ss], qp[:, :SBLK], Relu, bias=bq_sb[:]
                )

            def q_proj_sched(sb, slots):
                # split into matmul halves + relu, spaced so nothing waits
                # at its engine-queue head
                ss = ds(sb * SBLK, SBLK)
                xq_st = stage.tile([128, NKC, 2, SBLK], fp8, tag="xq", name="xq_st")
                nc.sync.dma_start(xq_st[:], xq_r[sb])
                st = {}

                def mm(c0, c1):
                    def f():
                        if "qp" not in st:
                            st["qp"] = ps_av.tile(
                                [128, 1024], f32, tag=f"av{sb % 2}", bufs=1, name="qp"
                            )
                        for kc in range(c0, c1):
                            nc.tensor.matmul(
                                st["qp"][:, :SBLK], wq_sb[:, kc, :, :],
                                xq_st[:, kc, :, :],
                                start=(kc == 0), stop=(kc == NKC - 1),
                                perf_mode=DR,
                            )
                    return f

                def rl():
                    nc.scalar.activation(
                        qT_sb[:, ss], st["qp"][:, :SBLK], Relu, bias=bq_sb[:]
                    )

                side_sched.extend(
                    [[slots[0], mm(0, 2)], [slots[1], mm(2, 4)], [slots[2], rl]]
                )

            def scores_exp(sb, ti, ex4):
                # ex4: [128, head, i, 512] e4m3 tile for chunk ti//2
                ss = ds(sb * SBLK, SBLK)
                tt = ds(ti * 128, 128)
                i = ti % 2
                scd = ps_d.tile([128, 512], f32, tag="d", name="scd")
                scs = ps_s.tile([128, 512], f32, tag="s", name="scs")
                nc.tensor.matmul(
                    scd[:], kT_sb[0:64, tt], qT_sb[0:64, ss],
                    start=True, stop=True,
                )
                nc.tensor.matmul(
                    scs[:], kT_sb[64:128, tt], qT_sb[64:128, ss],
                    start=True, stop=True,
                )
                nc.vector.tensor_scalar(
                    ex4.bitcast(i8)[:, 0, i, :], scd[:],
                    SCH8_MUL, SCH8_ADD, mult, add,
                )
                nc.scalar.activation(
                    ex4[:, 1, i, :], scs[:], Exp, scale=0.125
                )

            def av_mm(av, c, ex4):
                for hl in range(2):
                    nc.tensor.matmul(
                        av[0:65, ts(hl, SBLK)],
                        v_sb[:, c, hl, :, 0:65],
                        ex4[:, hl, :, :],
                        start=(c == 0), stop=(c == NC2 - 1),
                        perf_mode=DR,
                    )

            pend = []  # deferred (chunk, ex4) AV inputs
            cur = []  # ex4 tile being filled (allocated at even ti)

            def attn_ti(sb, av, ti):
                if ti % 2 == 0:
                    cur.append(expp.tile([128, 2, 2, 512], fp8, name="ex4"))
                ex4 = cur[-1]
                scores_exp(sb, ti, ex4)
                if ti % 2 == 1:
                    # chunk complete; keep AV one chunk behind the scores so
                    # the AV matmuls never wait on freshly-produced ex
                    if pend:
                        pc, pex = pend.pop(0)
                        av_mm(av, pc, pex)
                    pend.append((ti // 2, ex4))
                    cur.clear()
                    if ti == NT - 1:
                        for pc, pex in pend:
                            av_mm(av, pc, pex)
                        pend.clear()
                # slot-scheduled side work (epilogue pieces, next q_proj)
                side_sched.sort(key=lambda x: x[0])
                while side_sched and ti >= side_sched[0][0]:
                    side_sched.pop(0)[1]()

            def epilogue(sb, av, last=False):
                # normalize + residual + store; split into side-tasks that
                # interleave with the next s-block's iterations. All pieces
                # are scheduled LATE so no engine FIFO head-blocks on the
                # den chain (copy -> DMA -> gpsimd bcast).
                j = sb // 2
                mm = ds((sb % 2) * SBLK, SBLK)
                qrt = epi.tile([64, 1024], bf16, name="qrt")
                for hl in range(2):
                    nc.gpsimd.dma_start(
                        qrt[:, ts(hl, SBLK)], qres_r[hl, :, j, mm]
                    )
                d64 = epi.tile([65, 1024], f32, name="d64")
                d0b = epi.tile([65, 1024], bf16, name="d0b")
                den0 = epi.tile([1, 1024], f32, name="den0")
                bcd = epi.tile([64, 1024], f32, name="bcd")
                bcs = epi.tile([64, 1024], f32, name="bcs")
                prod = epi.tile([64, 1024], bf16, name="prod")
                outt = epi.tile([64, 1024], bf16, name="outt")

                def dn():
                    # den row (partition 64) -> SBUF -> partition 0 -> bcast
                    if not last:
                        nc.scalar.copy(d64[64:65, :], av[64:65, :])
                        nc.sync.dma_start(den0[:], d64[64:65, :])
                        nc.gpsimd.partition_broadcast(bcd[:], den0[:])
                        return
                    # tail: avoid the slow tail gpsimd broadcast -- PE K=1
                    # matmuls broadcast the bf16 den row into the now-idle
                    # score banks; ScalarE copies it back to SBUF for the
                    # (SBUF-source-only) fast reciprocal
                    nc.scalar.copy(d0b[64:65, :], av[64:65, :])
                    bct_d = ps_d.tile([128, 512], f32, tag="d", name="bct_d")
                    bct_s = ps_s.tile([128, 512], f32, tag="s", name="bct_s")
                    for hl, bt in ((0, bct_d), (1, bct_s)):
                        nc.tensor.matmul(
                            bt[0:64, :], ones64[64:65, :],
                            d0b[64:65, ts(hl, SBLK)],
                            start=True, stop=True,
                        )
                    nc.scalar.copy(bcd[:, 0:512], bct_d[0:64, :])
                    nc.scalar.copy(bcd[:, 512:1024], bct_s[0:64, :])

                def rc():
                    nc.vector.reciprocal_approx_fast(bcs[:], bcd[:])

                def pr():
                    nc.vector.tensor_tensor(
                        prod[:], av[0:64, :], bcs[:], mult
                    )

                def fin():
                    eng = nc.vector if last else nc.gpsimd
                    eng.tensor_tensor(outt[:], prod[:], qrt[:], add)
                    for hl in range(2):
                        nc.gpsimd.dma_start(
                            out_r[hl, :, j, mm], outt[:, ts(hl, SBLK)]
                        )

                side_sched.extend([[1, dn], [9, rc], [11, pr], [13, fin]])

            # ---- k/v projections interleaved with attention(0) ----
            av0 = ps_av.tile([128, 1024], f32, tag="av0", bufs=1, name="av")
            # kp lives in the av1 bank-pair, idle until s-block 1 starts
            av1_wu = ps_av.tile([128, 1024], f32, tag="av1", bufs=1, name="av1_wu")
            kp = av1_wu[:, 512:1024]

            def k_proj(sb, xk_st):
                # k projection runs one s-chunk AHEAD of attention(0) so
                # the k-relu is never on the scores' critical path
                for kc in range(NKC):
                    nc.tensor.matmul(
                        kp, wk_sb[:, kc, :, :], xk_st[:, kc, :, :],
                        start=(kc == 0), stop=(kc == NKC - 1), perf_mode=DR,
                    )
                nc.scalar.activation(
                    kT_sb[:, ds(sb * SBLK, SBLK)], kp, Relu, bias=bk_sb[:]
                )

            def dma_xk(sb):
                xk_st = stage.tile([128, NKC, 2, SBLK], fp8, tag="xk", name="xk_st")
                nc.sync.dma_start(xk_st[:], xk_r[sb])
                return xk_st

            def dma_xv(sb):
                xv_st = stage.tile([128, NKCV, SBLK], bf16, tag="xv", name="xv_st")
                nc.sync.dma_start(xv_st[:], xv_r[sb])
                return xv_st

            xk_cur = dma_xk(0)
            xv_cur = dma_xv(0)
            nc.sync.dma_start(wv_sb[:], wv_r)
            nc.sync.dma_start(wq_sb[:], wq_r)
            k_proj(0, xk_cur)
            for sb in range(NSB):
                if sb + 1 < NSB:
                    xk_nxt = dma_xk(sb + 1)
                    xv_nxt = dma_xv(sb + 1)
                for tj in range(4):
                    ti = sb * 4 + tj
                    vp = av1_wu[:, 0:512]
                    for ci in range(NKCV):
                        nc.tensor.matmul(
                            vp[:, 0:128], xv_cur[:, ci, ts(tj, 128)], wv_sb[:, ci, :],
                            start=(ci == 0), stop=False,
                        )
                    nc.tensor.matmul(
                        vp[:, 0:128], ones_rowb[:1, :], bv_sb[:1, :],
                        start=False, stop=True,
                    )
                    nc.vector.tensor_scalar_max(
                        v_sb[:, ti // 2, :, ti % 2, 0:64],
                        vp[:, 0:128].rearrange("p (h w) -> p h w", h=2),
                        0.0,
                    )
                    if sb == 0 and tj == 0:
                        q_proj(0)
                    if tj == 2 and sb + 1 < NSB:
                        k_proj(sb + 1, xk_nxt)
                    attn_ti(0, av0, ti)
                if sb + 1 < NSB:
                    xk_cur, xv_cur = xk_nxt, xv_nxt
            epilogue(0, av0)

            # ---- remaining attention s-blocks ----
            q_proj(1)
            for sb in range(1, NSB):
                if sb + 1 < NSB:
                    q_proj_sched(sb + 1, (12, 13, 14))
                av = ps_av.tile([128, 1024], f32, tag=f"av{sb % 2}", bufs=1, name="av")
                for ti in range(NT):
                    attn_ti(sb, av, ti)
                epilogue(sb, av, last=(sb == NSB - 1))
            for _, fn in sorted(side_sched, key=lambda x: x[0]):
                fn()
            side_sched.clear()

    nc.compile()
    return nc


def _get_nc():
    if "nc" not in _CACHE:
        _CACHE["nc"] = _build_nc()
    return _CACHE["nc"]


def _arr_x8(x2):
    # [S, H] -> [sb, p, kc, i, s_local] fp8, chan = 256*kc + 128*i + p
    xT = x2.T.astype(FP8)  # [H, S]
    return np.ascontiguousarray(
        xT.reshape(NKC, 2, 128, NSB, SBLK).transpose(3, 2, 0, 1, 4)
    )


def _arr_xv(x2):
    # [S, H] -> [sb, p, c, s_local] bf16, chan = 128*c + p
    xT = x2.T.astype(BF16)  # [H, S]
    return np.ascontiguousarray(
        xT.reshape(NKCV, 128, NSB, SBLK).transpose(2, 1, 0, 3)
    )


def _arr_w8(wT):
    # [H, OC] -> [p, kc, i, o] fp8
    return np.ascontiguousarray(
        wT.astype(FP8).reshape(NKC, 2, 128, OC).transpose(2, 0, 1, 3)
    )


def _arr_wv(wT):
    # [H, OC] -> [p, c, o] bf16
    return np.ascontiguousarray(
        wT.astype(BF16).reshape(NKCV, 128, OC).transpose(1, 0, 2)
    )


def kernel(queries, keys, values, Wq_w, Wq_b, Wk_w, Wk_b, Wv_w, Wv_b, **kw):
    nc = _get_nc()
    q2 = np.asarray(queries, np.float32).reshape(S, H)
    k2 = np.asarray(keys, np.float32).reshape(S, H)
    v2 = np.asarray(values, np.float32).reshape(S, H)
    xq4 = _arr_x8(q2)
    xk4 = _arr_x8(k2)
    xv4 = _arr_xv(v2)

    in_maps = []
    for c in range(NCORES):
        o = slice(OC * c, OC * (c + 1))
        in_maps.append(
            {
                "xq": xq4,
                "xk": xk4,
                "xv": xv4,
                "wq": _arr_w8(np.asarray(Wq_w)[o].T),
                "wk": _arr_w8(np.asarray(Wk_w)[o].T),
                "wv": _arr_wv(np.asarray(Wv_w)[o].T),
                "bq": np.asarray(Wq_b, np.float32)[o].reshape(OC, 1),
                "bk": np.asarray(Wk_b, np.float32)[o].reshape(OC, 1),
                "bv": np.asarray(Wv_b)[o].astype(BF16).reshape(1, OC),
                "qres": np.ascontiguousarray(
                    q2[512 * c : 512 * (c + 1)]
                ).astype(BF16),
            }
        )

    res = run_bass_kernel_spmd(
        nc, in_maps, list(range(NCORES)), **_CACHE.get("run_kwargs", {})
    )
    _CACHE["last_results"] = res
    full = np.concatenate(
        [np.asarray(res.results[c]["out"], np.float32) for c in range(NCORES)],
        axis=0,
    )
    return full.reshape(1, S, H)


# revision 39
# speedup vs baseline: 1.1847x; 1.0263x over previous
"""Trainium2 Bass kernel for a 16-head MHA block (B=1, S=4096, H=1024).

Sharding: tensor-parallel over heads — each of the 8 cores owns 2 heads
(128 of the 1024 Wq/Wk/Wv output channels) and computes 512 rows of the
final (scrambled) output; the host concatenates the row blocks.

Per-core dataflow:
  qT/kT = relu(W @ x.T + b)      fp8e4m3 DoubleRow matmuls (K=256/chunk),
                                 bf16 [128 chan, 4096 seq] outputs
  v     = relu(x @ W.T + b)      bf16 matmuls, stored e4m3 in DoubleRow
                                 layout [128 t, chunk16, head2, par2, 80]
  S_T[t,s] = sum_d kT[d,t] qT[d,s]   bf16, 2 heads row-tiled (T0/T8, K=64
                                     each) so both matmuls run concurrently
  E = exp(S_T/8) in fp8e4m3, split across engines BY HEAD:
      head0: VectorE Schraudolph bit-trick (fp32 PSUM -> int8 e4m3 bits)
      head1: ScalarE exact Exp with fp8 output
  num/den = sum_t [v|1][t,d'] E[t,s]   fp8 DoubleRow matmul, K=256 virtual
            (t-chunk pairs), row 64 of v = 1.0 gives the denominator
  epilogue: den copy on ScalarE, recip + normalize + residual on DVE,
  partition-broadcast on GpSimd, interleaved as side-tasks into the
  next s-block's iteration loop.
"""

import math

import numpy as np
import ml_dtypes

import concourse.bass as bass
import concourse.tile as tile
from concourse import bacc, mybir
from concourse.bass import ds, ts
from concourse.bass_utils import run_bass_kernel_spmd

BF16 = ml_dtypes.bfloat16
FP8 = ml_dtypes.float8_e4m3
S = 4096
H = 1024
NCORES = 8
OC = H // NCORES  # 128 output channels (2 heads) per core
SBLK = 512  # s-block width
NSB = S // SBLK  # 8
NT = S // 128  # 32 t-chunks of 128
NC2 = NT // 2  # 16 DoubleRow t-chunks of 256
NKC = H // 256  # 4 DoubleRow contraction chunks for q/k projections
NKCV = H // 128  # 8 plain contraction chunks for the v projection

# Schraudolph exp for the DVE share, to e4m3 bits: bits of exp(s/8) are
# approx round(s/ln2 + 8*(7 - 0.0437)); scores >= 0 (post-relu q,k).
SCH8_MUL = 1.0 / math.log(2.0)
SCH8_ADD = 8.0 * (7.0 - 0.0437)

_CACHE = {}


def _build_nc():
    f32 = mybir.dt.float32
    bf16 = mybir.dt.bfloat16
    fp8 = mybir.dt.float8e4
    i8 = mybir.dt.int8
    add = mybir.AluOpType.add
    mult = mybir.AluOpType.mult
    Exp = mybir.ActivationFunctionType.Exp
    Relu = mybir.ActivationFunctionType.Relu
    DR = mybir.MatmulPerfMode.DoubleRow

    nc = bacc.Bacc("TRN2", target_bir_lowering=False, debug=False)

    # inputs are host-pre-arranged so every DMA is contiguous per partition:
    # xq/xk: [sb, p, kc, i, s_local] fp8 (chan = 256*kc + 128*i + p)
    # xv:    [sb, p, c, s_local] bf16 (chan = 128*c + p)
    # wq/wk: [p, kc, i, o] fp8 ; wv: [p, c, o] bf16
    xq_r = nc.dram_tensor(
        "xq", [NSB, 128, NKC, 2, SBLK], fp8, kind="ExternalInput"
    ).ap()
    xk_r = nc.dram_tensor(
        "xk", [NSB, 128, NKC, 2, SBLK], fp8, kind="ExternalInput"
    ).ap()
    xv_r = nc.dram_tensor(
        "xv", [NSB, 128, NKCV, SBLK], bf16, kind="ExternalInput"
    ).ap()
    wq_r = nc.dram_tensor("wq", [128, NKC, 2, OC], fp8, kind="ExternalInput").ap()
    wk_r = nc.dram_tensor("wk", [128, NKC, 2, OC], fp8, kind="ExternalInput").ap()
    wv_r = nc.dram_tensor("wv", [128, NKCV, OC], bf16, kind="ExternalInput").ap()
    bq = nc.dram_tensor("bq", [OC, 1], f32, kind="ExternalInput").ap()
    bk = nc.dram_tensor("bk", [OC, 1], f32, kind="ExternalInput").ap()
    bv = nc.dram_tensor("bv", [1, OC], bf16, kind="ExternalInput").ap()
    qres = nc.dram_tensor("qres", [512, H], bf16, kind="ExternalInput").ap()
    out = nc.dram_tensor("out", [512, H], bf16, kind="ExternalOutput").ap()
    # residual/output rows: local row = 256*hl + 4*d + j
    qres_r = qres.rearrange("(hl d j) m -> hl d j m", hl=2, d=64)
    out_r = out.rearrange("(hl d j) m -> hl d j m", hl=2, d=64)

    with tile.TileContext(nc) as tc:
        with (
            tc.tile_pool(name="const", bufs=1) as constp,
            tc.tile_pool(name="persist", bufs=1) as persist,
            tc.tile_pool(name="stage", bufs=2) as stage,
            tc.tile_pool(name="exps", bufs=6) as expp,
            tc.tile_pool(name="epi", bufs=2) as epi,
            tc.tile_pool(name="ps_d", bufs=2, space="PSUM") as ps_d,
            tc.tile_pool(name="ps_s", bufs=2, space="PSUM") as ps_s,
            tc.tile_pool(name="ps_av", bufs=2, space="PSUM") as ps_av,
        ):
            # ---- constants ----
            wq_sb = constp.tile([128, NKC, 2, OC], fp8)
            wk_sb = constp.tile([128, NKC, 2, OC], fp8)
            wv_sb = constp.tile([128, NKCV, OC], bf16)
            nc.sync.dma_start(wk_sb[:], wk_r)
            bq_sb = constp.tile([OC, 1], f32)
            bk_sb = constp.tile([OC, 1], f32)
            bv_sb = constp.tile([1, OC], bf16)
            nc.scalar.dma_start(bq_sb[:], bq)
            nc.scalar.dma_start(bk_sb[:], bk)
            nc.scalar.dma_start(bv_sb[:], bv)
            ones_rowb = constp.tile([1, 128], bf16)
            nc.vector.memset(ones_rowb[:], 1.0)
            ones64 = constp.tile([65, 64], bf16)
            nc.vector.memset(ones64[64:65, :], 1.0)

            qT_sb = persist.tile([128, S], bf16)
            kT_sb = persist.tile([128, S], bf16)
            # v in fp8 DoubleRow layout: [p, chunk, head, i, 80]
            # element = v[t = 256*chunk + 128*i + p, 64*head + d]; d=64 is
            # the ones column (denominator row of the AV matmul).
            v_sb = persist.tile([128, NC2, 2, 2, 80], fp8)
            nc.vector.memset(v_sb[:, :, :, :, 64:65], 1.0)

            # ---- helper defs ----
            side_sched = []  # [slot, fn]: fn runs at first iteration >= slot

            def q_proj(sb):
                ss = ds(sb * SBLK, SBLK)
                xq_st = stage.tile([128, NKC, 2, SBLK], fp8, tag="xq", name="xq_st")
                nc.sync.dma_start(xq_st[:], xq_r[sb])
                # qp lives in the av bank-pair of the s-block being
                # epilogued (dead by the time q_proj runs)
                qp = ps_av.tile([128, 1024], f32, tag=f"av{sb % 2}", bufs=1, name="qp")
                for kc in range(NKC):
                    nc.tensor.matmul(
                        qp[:, :SBLK], wq_sb[:, kc, :, :], xq_st[:, kc, :, :],
                        start=(kc == 0), stop=(kc == NKC - 1), perf_mode=DR,
                    )
                nc.scalar.activation(
                    qT_sb[:, ss], qp[:, :SBLK], Relu, bias=bq_sb[:]
                )

            def q_proj_sched(sb, slots):
                # split into matmul halves + relu, spaced so nothing waits
                # at its engine-queue head
                ss = ds(sb * SBLK, SBLK)
                xq_st = stage.tile([128, NKC, 2, SBLK], fp8, tag="xq", name="xq_st")
                nc.sync.dma_start(xq_st[:], xq_r[sb])
                st = {}

                def mm(c0, c1):
                    def f():
                        if "qp" not in st:
                            st["qp"] = ps_av.tile(
                                [128, 1024], f32, tag=f"av{sb % 2}", bufs=1, name="qp"
                            )
                        for kc in range(c0, c1):
                            nc.tensor.matmul(
                                st["qp"][:, :SBLK], wq_sb[:, kc, :, :],
                                xq_st[:, kc, :, :],
                                start=(kc == 0), stop=(kc == NKC - 1),
                                perf_mode=DR,
                            )
                    return f

                def rl():
                    nc.scalar.activation(
                        qT_sb[:, ss], st["qp"][:, :SBLK], Relu, bias=bq_sb[:]
                    )

                side_sched.extend(
                    [[slots[0], mm(0, 2)], [slots[1], mm(2, 4)], [slots[2], rl]]
                )

            def scores_exp(sb, ti, ex4):
                # ex4: [128, head, i, 512] e4m3 tile for chunk ti//2
                ss = ds(sb * SBLK, SBLK)
                tt = ds(ti * 128, 128)
                i = ti % 2
                scd = ps_d.tile([128, 512], f32, tag="d", name="scd")
                scs = ps_s.tile([128, 512], f32, tag="s", name="scs")
                nc.tensor.matmul(
                    scd[:], kT_sb[0:64, tt], qT_sb[0:64, ss],
                    start=True, stop=True,
                )
                nc.tensor.matmul(
                    scs[:], kT_sb[64:128, tt], qT_sb[64:128, ss],
                    start=True, stop=True,
                )
                nc.vector.tensor_scalar(
                    ex4.bitcast(i8)[:, 0, i, :], scd[:],
                    SCH8_MUL, SCH8_ADD, mult, add,
                )
                nc.scalar.activation(
                    ex4[:, 1, i, :], scs[:], Exp, scale=0.125
                )

            def av_mm(av, c, ex4):
                for hl in range(2):
                    nc.tensor.matmul(
                        av[0:65, ts(hl, SBLK)],
                        v_sb[:, c, hl, :, 0:65],
                        ex4[:, hl, :, :],
                        start=(c == 0), stop=(c == NC2 - 1),
                        perf_mode=DR,
                    )

            pend = []  # deferred (chunk, ex4) AV inputs
            cur = []  # ex4 tile being filled (allocated at even ti)

            def attn_ti(sb, av, ti):
                if ti % 2 == 0:
                    cur.append(expp.tile([128, 2, 2, 512], fp8, name="ex4"))
                ex4 = cur[-1]
                scores_exp(sb, ti, ex4)
                if ti % 2 == 1:
                    # chunk complete; keep AV one chunk behind the scores so
                    # the AV matmuls never wait on freshly-produced ex
                    if pend:
                        pc, pex = pend.pop(0)
                        av_mm(av, pc, pex)
                    pend.append((ti // 2, ex4))
                    cur.clear()
                    if ti == NT - 1:
                        for pc, pex in pend:
                            av_mm(av, pc, pex)
                        pend.clear()
                # slot-scheduled side work (epilogue pieces, next q_proj)
                side_sched.sort(key=lambda x: x[0])
                while side_sched and ti >= side_sched[0][0]:
                    side_sched.pop(0)[1]()

            def epilogue(sb, av, last=False):
                # normalize + residual + store; split into side-tasks that
                # interleave with the next s-block's iterations. All pieces
                # are scheduled LATE so no engine FIFO head-blocks on the
                # den chain (copy -> DMA -> gpsimd bcast).
                j = sb // 2
                mm = ds((sb % 2) * SBLK, SBLK)
                qrt = epi.tile([64, 1024], bf16, name="qrt")
                for hl in range(2):
                    nc.gpsimd.dma_start(
                        qrt[:, ts(hl, SBLK)], qres_r[hl, :, j, mm]
                    )
                d64 = epi.tile([65, 1024], f32, name="d64")
                d0b = epi.tile([65, 1024], bf16, name="d0b")
                den0 = epi.tile([1, 1024], f32, name="den0")
                bcd = epi.tile([64, 1024], f32, name="bcd")
                bcs = epi.tile([64, 1024], f32, name="bcs")
                prod = epi.tile([64, 1024], bf16, name="prod")
                outt = epi.tile([64, 1024], bf16, name="outt")

                def dn():
                    # den row (partition 64) -> SBUF -> partition 0 -> bcast
                    if not last:
                        nc.scalar.copy(d64[64:65, :], av[64:65, :])
                        nc.sync.dma_start(den0[:], d64[64:65, :])
                        nc.gpsimd.partition_broadcast(bcd[:], den0[:])
                        return
                    # tail: avoid the slow tail gpsimd broadcast -- PE K=1
                    # matmuls broadcast the bf16 den row into the now-idle
                    # score banks; ScalarE copies it back to SBUF for the
                    # (SBUF-source-only) fast reciprocal
                    nc.scalar.copy(d0b[64:65, :], av[64:65, :])
                    bct_d = ps_d.tile([128, 512], f32, tag="d", name="bct_d")
                    bct_s = ps_s.tile([128, 512], f32, tag="s", name="bct_s")
                    for hl, bt in ((0, bct_d), (1, bct_s)):
                        nc.tensor.matmul(
                            bt[0:64, :], ones64[64:65, :],
                            d0b[64:65, ts(hl, SBLK)],
                            start=True, stop=True,
                        )
                    nc.scalar.copy(bcd[:, 0:512], bct_d[0:64, :])
                    nc.scalar.copy(bcd[:, 512:1024], bct_s[0:64, :])

                def rc():
                    nc.vector.reciprocal_approx_fast(bcs[:], bcd[:])

                def pr():
                    nc.vector.tensor_tensor(
                        prod[:], av[0:64, :], bcs[:], mult
                    )

                def fin():
                    eng = nc.vector if last else nc.gpsimd
                    eng.tensor_tensor(outt[:], prod[:], qrt[:], add)
                    for hl in range(2):
                        nc.gpsimd.dma_start(
                            out_r[hl, :, j, mm], outt[:, ts(hl, SBLK)]
                        )

                side_sched.extend([[1, dn], [9, rc], [11, pr], [13, fin]])

            # ---- k/v projections interleaved with attention(0) ----
            av0 = ps_av.tile([128, 1024], f32, tag="av0", bufs=1, name="av")
            # kp lives in the av1 bank-pair, idle until s-block 1 starts
            av1_wu = ps_av.tile([128, 1024], f32, tag="av1", bufs=1, name="av1_wu")
            kp = av1_wu[:, 512:1024]

            def k_proj(sb, xk_st):
                # k projection runs one s-chunk AHEAD of attention(0) so
                # the k-relu is never on the scores' critical path
                for kc in range(NKC):
                    nc.tensor.matmul(
                        kp, wk_sb[:, kc, :, :], xk_st[:, kc, :, :],
                        start=(kc == 0), stop=(kc == NKC - 1), perf_mode=DR,
                    )
                nc.scalar.activation(
                    kT_sb[:, ds(sb * SBLK, SBLK)], kp, Relu, bias=bk_sb[:]
                )

            def dma_xk(sb):
                xk_st = stage.tile([128, NKC, 2, SBLK], fp8, tag="xk", name="xk_st")
                nc.sync.dma_start(xk_st[:], xk_r[sb])
                return xk_st

            def dma_xv(sb):
                xv_st = stage.tile([128, NKCV, SBLK], bf16, tag="xv", name="xv_st")
                nc.sync.dma_start(xv_st[:], xv_r[sb])
                return xv_st

            xk_cur = dma_xk(0)
            xv_cur = dma_xv(0)
            nc.sync.dma_start(wv_sb[:], wv_r)
            nc.sync.dma_start(wq_sb[:], wq_r)
            k_proj(0, xk_cur)
            for sb in range(NSB):
                if sb + 1 < NSB:
                    xk_nxt = dma_xk(sb + 1)
                    xv_nxt = dma_xv(sb + 1)
                for tj in range(4):
                    ti = sb * 4 + tj
                    vp = ps_d.tile([128, 512], f32, tag="d", name="vp")
                    for ci in range(NKCV):
                        nc.tensor.matmul(
                            vp[:, 0:128], xv_cur[:, ci, ts(tj, 128)], wv_sb[:, ci, :],
                            start=(ci == 0), stop=False,
                        )
                    nc.tensor.matmul(
                        vp[:, 0:128], ones_rowb[:1, :], bv_sb[:1, :],
                        start=False, stop=True,
                    )
                    nc.vector.tensor_scalar_max(
                        v_sb[:, ti // 2, :, ti % 2, 0:64],
                        vp[:, 0:128].rearrange("p (h w) -> p h w", h=2),
                        0.0,
                    )
                    if sb == 0 and tj == 0:
                        q_proj(0)
                    if tj == 2 and sb + 1 < NSB:
                        k_proj(sb + 1, xk_nxt)
                    attn_ti(0, av0, ti)
                if sb + 1 < NSB:
                    xk_cur, xv_cur = xk_nxt, xv_nxt
            epilogue(0, av0)

            # ---- remaining attention s-blocks ----
            q_proj(1)
            for sb in range(1, NSB):
                if sb + 1 < NSB:
                    q_proj_sched(sb + 1, (12, 13, 14))
                av = ps_av.tile([128, 1024], f32, tag=f"av{sb % 2}", bufs=1, name="av")
                for ti in range(NT):
                    attn_ti(sb, av, ti)
                epilogue(sb, av, last=(sb == NSB - 1))
            for _, fn in sorted(side_sched, key=lambda x: x[0]):
                fn()
            side_sched.clear()

    nc.compile()
    return nc


def _get_nc():
    if "nc" not in _CACHE:
        _CACHE["nc"] = _build_nc()
    return _CACHE["nc"]


def _arr_x8(x2):
    # [S, H] -> [sb, p, kc, i, s_local] fp8, chan = 256*kc + 128*i + p
    xT = x2.T.astype(FP8)  # [H, S]
    return np.ascontiguousarray(
        xT.reshape(NKC, 2, 128, NSB, SBLK).transpose(3, 2, 0, 1, 4)
    )


def _arr_xv(x2):
    # [S, H] -> [sb, p, c, s_local] bf16, chan = 128*c + p
    xT = x2.T.astype(BF16)  # [H, S]
    return np.ascontiguousarray(
        xT.reshape(NKCV, 128, NSB, SBLK).transpose(2, 1, 0, 3)
    )


def _arr_w8(wT):
    # [H, OC] -> [p, kc, i, o] fp8
    return np.ascontiguousarray(
        wT.astype(FP8).reshape(NKC, 2, 128, OC).transpose(2, 0, 1, 3)
    )


def _arr_wv(wT):
    # [H, OC] -> [p, c, o] bf16
    return np.ascontiguousarray(
        wT.astype(BF16).reshape(NKCV, 128, OC).transpose(1, 0, 2)
    )


def kernel(queries, keys, values, Wq_w, Wq_b, Wk_w, Wk_b, Wv_w, Wv_b, **kw):
    nc = _get_nc()
    q2 = np.asarray(queries, np.float32).reshape(S, H)
    k2 = np.asarray(keys, np.float32).reshape(S, H)
    v2 = np.asarray(values, np.float32).reshape(S, H)
    xq4 = _arr_x8(q2)
    xk4 = _arr_x8(k2)
    xv4 = _arr_xv(v2)

    in_maps = []
    for c in range(NCORES):
        o = slice(OC * c, OC * (c + 1))
        in_maps.append(
            {
                "xq": xq4,
                "xk": xk4,
                "xv": xv4,
                "wq": _arr_w8(np.asarray(Wq_w)[o].T),
                "wk": _arr_w8(np.asarray(Wk_w)[o].T),
                "wv": _arr_wv(np.asarray(Wv_w)[o].T),
                "bq": np.asarray(Wq_b, np.float32)[o].reshape(OC, 1),
                "bk": np.asarray(Wk_b, np.float32)[o].reshape(OC, 1),
                "bv": np.asarray(Wv_b)[o].astype(BF16).reshape(1, OC),
                "qres": np.ascontiguousarray(
                    q2[512 * c : 512 * (c + 1)]
                ).astype(BF16),
            }
        )

    res = run_bass_kernel_spmd(
        nc, in_maps, list(range(NCORES)), **_CACHE.get("run_kwargs", {})
    )
    _CACHE["last_results"] = res
    full = np.concatenate(
        [np.asarray(res.results[c]["out"], np.float32) for c in range(NCORES)],
        axis=0,
    )
    return full.reshape(1, S, H)
